# revision 25
# baseline (speedup 1.0000x reference)
"""Trainium2 Bass kernel for nn_Attention_20976620274235 (sparse attention).

Key idea: vis_mask rows/cols that are masked out contribute exactly zero to
the output (masked q rows give attn=0 -> out row 0; masked k positions are
excluded from the softmax).  So we COMPACT: host gathers the ~S/2 visible
positions per batch, pads to SPAD=1152 (=9*128, +5.7 sigma above the
Binomial(2048,.5) mean), the device computes attention on the short
sequence, and the host scatters rows back (zeros elsewhere).

Sharding: 8 cores = 4 batches x 2 head-groups (8 heads each).
Per-core SPMD program (all fp16 matmuls, fp32 PSUM):
  1. k-head projections + RoPE (q/k SBUF-resident, head-dim-major [hd, s])
  2. V projection (s-major fp16)
  3. per head h: q-head projection, then attention with TRANSPOSED scores
     sT[k, q] = kT.T @ qT (padded-column mask = per-partition bias on the
     Exp activation; P^T feeds P@V directly as the moving operand).
     The attention t-loop is k-tile-outer so each kT / V stationary is
     loaded once per (h, t) and reused across the three q-chunks, and
     scores run one k-tile ahead of PV so the scalar-engine Exp latency
     stays off the PE critical path.  Unnormalized accumulate; divide by
     (ones^T @ P^T) at the end.  Interleaving per-head QKV with attention
     gives the scalar engine PE-only stretches to catch up in.
  4. output projection, partial over this core's 1024 channels.
Host: sums the two head-group partials per batch, scatters visible rows.

Engine assignment: PE matmuls; scalar engine does Exp (and PSUM copies
only in attention-free stretches); DVE does RoPE muls/adds, reciprocal,
final scale.  One PSUM plan for the whole kernel: shared ps pool (3) +
po (3) + pd (1) + rot (1) = 8 banks.
"""

import math

import numpy as np

import concourse.bass as bass
from concourse import bacc
import concourse.mybir as mybir
import concourse.tile as tile
from concourse.bass_utils import run_bass_kernel_spmd

B, S, DIM, H = 4, 2048, 2048, 16
HD = 128          # head dim
NC = 8            # cores
HC = 8            # heads per core
CC = HC * HD      # 1024 channels per core
SPAD = 1152       # padded compacted sequence length (9 * 128)
F32 = mybir.dt.float32
F16 = mybir.dt.float16
SM_SCALE = 1.0 / math.sqrt(HD)
EXP_BIAS = -6.0   # shift-invariant; keeps exp() in f16 normal range
MASK_BIAS = -1.0e9

_CACHE = {}


def _build_program(spad):
    assert spad == 1152
    nt = spad // 128   # 9 k-tiles
    TQW = spad - 1024  # 128-wide tail q-chunk

    nc = bacc.Bacc("TRN2", target_bir_lowering=False, debug=False, num_devices=NC)

    # host-pretiled inputs: layouts match SBUF exactly (contiguous DMAs)
    xg = nc.dram_tensor("xg", [128, 16 * spad], F16, kind="ExternalInput").ap()
    wqk = nc.dram_tensor("wqk", [128, 16 * 16 * 128], F16, kind="ExternalInput").ap()
    wv = nc.dram_tensor("wv", [128, 16 * CC], F16, kind="ExternalInput").ap()
    wo = nc.dram_tensor("wo", [128, 8 * DIM], F16, kind="ExternalInput").ap()
    cosg = nc.dram_tensor("cosg", [HD, spad], F16, kind="ExternalInput").ap()
    sing = nc.dram_tensor("sing", [HD, spad], F16, kind="ExternalInput").ap()
    padc = nc.dram_tensor("padc", [1, 1], F32, kind="ExternalInput").ap()
    rotT = nc.dram_tensor("rotT", [HD, HD], F16, kind="ExternalInput").ap()
    out = nc.dram_tensor("out", [spad, DIM], F16, kind="ExternalOutput").ap()

    Exp = mybir.ActivationFunctionType.Exp

    with tile.TileContext(nc) as tc:
        with tc.tile_pool(name="consts", bufs=1) as cpool, \
             tc.tile_pool(name="persist", bufs=1) as ppool, \
             tc.tile_pool(name="xp", bufs=1) as xpool, \
             tc.tile_pool(name="qc", bufs=2) as qpool, \
             tc.tile_pool(name="wmp", bufs=2) as wmpool, \
             tc.tile_pool(name="rp", bufs=2) as rpool, \
             tc.tile_pool(name="ptp", bufs=4) as ptpool, \
             tc.tile_pool(name="smp", bufs=1) as smpool, \
             tc.tile_pool(name="obp", bufs=2) as obpool, \
             tc.tile_pool(name="pss", bufs=3, space="PSUM") as pss, \
             tc.tile_pool(name="pso", bufs=2, space="PSUM") as pso, \
             tc.tile_pool(name="psd", bufs=1, space="PSUM") as psd, \
             tc.tile_pool(name="psb", bufs=2, space="PSUM") as psb:
            cos_sb = cpool.tile([HD, spad], F16)
            sin_sb = cpool.tile([HD, spad], F16)
            pc_sb = cpool.tile([1, 1], F32)
            rt_sb = cpool.tile([HD, HD], F16)
            ones_sb = cpool.tile([128, 1], F16)
            onesr_sb = cpool.tile([1, 128], F16)
            eb_sb = cpool.tile([128, 1], F32)
            nc.gpsimd.memset(ones_sb[:], 1.0)
            nc.gpsimd.memset(onesr_sb[:], 1.0)
            nc.gpsimd.memset(eb_sb[:], EXP_BIAS)

            k_all = ppool.tile([128, 8 * spad], F16)    # [hd, kh*spad + pos]
            V_all = ppool.tile([128, nt * CC], F16)     # [s%128, j*CC + ch]
            OT_all = ppool.tile([128, HC * spad], F16)  # [hd, h*spad + pos]

            # x chunked per contraction tile; DMA only the first few chunks
            # before the first weight tile so nothing big blocks it
            x_t = []
            for t in range(16):
                xt = xpool.tile([128, spad], F16, tag=f"x{t}")
                x_t.append(xt)
            for t in range(3):
                nc.sync.dma_start(x_t[t][:], xg[:, t * spad:(t + 1) * spad])

            # RoPE chain runs one (m, chunk) behind the projection matmuls
            pending = [None]

            def flush_rope():
                if pending[0] is None:
                    return
                ps, dst, d0, c0, cw = pending[0]
                pending[0] = None
                qraw = rpool.tile([128, 512], F16, tag="qraw")
                nc.scalar.copy(qraw[:, :cw], ps[:, :cw])
                pr = psb.tile([128, 512], F32, tag="bc")
                nc.tensor.matmul(pr[:, :cw], lhsT=rt_sb[:], rhs=qraw[:, :cw],
                                 start=True, stop=True)
                t1 = rpool.tile([128, 512], F16, tag="t1")
                nc.vector.tensor_mul(t1[:, :cw], qraw[:, :cw],
                                     cos_sb[:, c0:c0 + cw])
                t2 = rpool.tile([128, 512], F16, tag="t2")
                nc.vector.tensor_mul(t2[:, :cw], pr[:, :cw],
                                     sin_sb[:, c0:c0 + cw])
                nc.vector.tensor_add(dst[:, d0:d0 + cw], t1[:, :cw], t2[:, :cw])

            def qk_project(m, dst, chunk_order, wm=None):
                if wm is None:
                    wm = wmpool.tile([128, 16 * 128], F16, tag="wm")
                    nc.sync.dma_start(wm[:], wqk[:, m * 2048:(m + 1) * 2048])
                for c0, cw in chunk_order:
                    ps = pss.tile([128, 512], F32, tag="sps")
                    for t in range(16):
                        nc.tensor.matmul(
                            ps[:, :cw],
                            lhsT=wm[:, t * 128:(t + 1) * 128],
                            rhs=x_t[t][:, c0:c0 + cw],
                            start=(t == 0), stop=(t == 15))
                    flush_rope()
                    pending[0] = (ps, dst, c0, c0, cw)

            CHUNKS = [(0, 512), (512, 512), (1024, TQW)]

            # ---- k-head projections (m 8..15), then V ----
            # weight tiles for m=8,9 queued before the bulk x DMAs; all x
            # DMAs are emitted before any matmul that reads them
            wm8 = wmpool.tile([128, 16 * 128], F16, tag="wm")
            nc.sync.dma_start(wm8[:], wqk[:, 8 * 2048: 9 * 2048])
            wm9 = wmpool.tile([128, 16 * 128], F16, tag="wm")
            nc.sync.dma_start(wm9[:], wqk[:, 9 * 2048: 10 * 2048])
            nc.sync.dma_start(rt_sb[:], rotT[:])
            nc.sync.dma_start(cos_sb[:], cosg[:])
            nc.sync.dma_start(sin_sb[:], sing[:])
            nc.sync.dma_start(pc_sb[:], padc[:])
            for t in range(3, 16):
                nc.sync.dma_start(x_t[t][:], xg[:, t * spad:(t + 1) * spad])
            qk_project(8, k_all[:, 0:spad], CHUNKS, wm=wm8)
            qk_project(9, k_all[:, spad:2 * spad], CHUNKS, wm=wm9)
            with tc.tile_pool(name="wvp", bufs=1) as wvpool:
                wv_sb = wvpool.tile([128, 16 * CC], F16)
                nc.sync.dma_start(wv_sb[:], wv[:])
                for m in range(10, 16):
                    qk_project(m, k_all[:, (m - 8) * spad:(m - 7) * spad], CHUNKS)
                for j in range(nt):  # V: out [pos, vch] s-major
                    for half in range(2):
                        pv = pss.tile([128, 512], F32, tag="sps")
                        for t in range(16):
                            nc.tensor.matmul(
                                pv[:],
                                lhsT=x_t[t][:, j * 128:(j + 1) * 128],
                                rhs=wv_sb[:, t * CC + half * 512: t * CC + (half + 1) * 512],
                                start=(t == 0), stop=(t == 15))
                        nc.scalar.copy(
                            V_all[:, j * CC + half * 512: j * CC + (half + 1) * 512],
                            pv[:])
            wo_sb = ppool.tile([128, 8 * DIM], F16)
            nc.sync.dma_start(wo_sb[:], wo[:])  # prefetch for output proj

            # ---- per head: q projection then attention ----
            for h in range(HC):
                q_t = qpool.tile([128, spad], F16, tag="qcur")
                # attention consumes c0 first; c1 is flushed at loop end
                qk_project(h, q_t, [CHUNKS[2], CHUNKS[0], CHUNKS[1]])
                flush_rope()
                kbase = h * spad

                def kT(t):
                    return k_all[:, kbase + t * 128: kbase + (t + 1) * 128]

                def vT(t):
                    return V_all[:, t * CC + h * 128: t * CC + (h + 1) * 128]

                def normalize(po, pd, c0, cw):
                    den = smpool.tile([1, 512], F32, tag="den")
                    nc.vector.tensor_scalar_sub(den[:, :cw], pd[0:1, :cw],
                                                pc_sb[:])
                    rec = smpool.tile([1, 512], F32, tag="rec")
                    nc.vector.reciprocal_approx_fast(rec[:, :cw], den[:, :cw])
                    rec16 = smpool.tile([1, 512], F16, tag="rec16")
                    nc.vector.tensor_copy(rec16[:, :cw], rec[:, :cw])
                    bcp = psb.tile([128, 512], F32, tag="bc")
                    nc.tensor.matmul(bcp[:, :cw], lhsT=onesr_sb[:],
                                     rhs=rec16[:, :cw], start=True, stop=True)
                    bcs = smpool.tile([128, 512], F16, tag="bcs")
                    nc.vector.tensor_copy(bcs[:, :cw], bcp[:, :cw])
                    nc.vector.tensor_mul(
                        OT_all[:, h * spad + c0: h * spad + c0 + cw],
                        po[:, :cw], bcs[:, :cw])

                # 512-wide q-chunks: per-t exp, scores 2 k-tiles ahead
                for c0, cw in CHUNKS[:2]:
                    po = pso.tile([128, 512], F32, tag="po")
                    pd = psd.tile([128, 512], F32, tag="pd")
                    pt_l = {}
                    for t in range(nt + 2):
                        if t < nt:
                            sp = pss.tile([128, 512], F32, tag="sps")
                            nc.tensor.matmul(
                                sp[:, :cw], lhsT=kT(t),
                                rhs=q_t[:, c0:c0 + cw],
                                start=True, stop=True)
                            pt = ptpool.tile([128, 512], F16, tag="pt")
                            nc.scalar.activation(pt[:, :cw], sp[:, :cw], Exp,
                                                 bias=eb_sb[:], scale=SM_SCALE)
                            pt_l[t] = pt
                        tt = t - 2
                        if 0 <= tt:
                            pt = pt_l.pop(tt)
                            nc.tensor.matmul(
                                po[:, :cw], lhsT=vT(tt), rhs=pt[:, :cw],
                                start=(tt == 0), stop=(tt == nt - 1))
                            nc.tensor.matmul(
                                pd[0:1, :cw], lhsT=ones_sb[:], rhs=pt[:, :cw],
                                start=(tt == 0), stop=(tt == nt - 1))
                    normalize(po, pd, c0, cw)

                # tail q-chunk (128 wide): batch 3 k-tiles per exp
                c0, cw = CHUNKS[2]
                po = pso.tile([128, 512], F32, tag="po")
                pd = psd.tile([128, 512], F32, tag="pd")
                pt_l = {}
                for g in range(4):  # groups of 3 k-tiles; one-ahead pipeline
                    if g < 3:
                        sp = pss.tile([128, 512], F32, tag="sps")
                        for i in range(3):
                            t = g * 3 + i
                            nc.tensor.matmul(
                                sp[:, i * 128:(i + 1) * 128], lhsT=kT(t),
                                rhs=q_t[:, c0:c0 + cw],
                                start=True, stop=True)
                        pt = ptpool.tile([128, 512], F16, tag="pt")
                        nc.scalar.activation(pt[:, :384], sp[:, :384], Exp,
                                             bias=eb_sb[:], scale=SM_SCALE)
                        pt_l[g] = pt
                    gg = g - 1
                    if 0 <= gg:
                        pt = pt_l.pop(gg)
                        for i in range(3):
                            t = gg * 3 + i
                            nc.tensor.matmul(
                                po[:, :cw], lhsT=vT(t),
                                rhs=pt[:, i * 128:(i + 1) * 128],
                                start=(t == 0), stop=(t == nt - 1))
                            nc.tensor.matmul(
                                pd[0:1, :cw], lhsT=ones_sb[:],
                                rhs=pt[:, i * 128:(i + 1) * 128],
                                start=(t == 0), stop=(t == nt - 1))
                normalize(po, pd, c0, cw)

            # ---- output projection ----
            for sj in range(nt):
                for oc in range(4):
                    pf = pss.tile([128, 512], F32, tag="sps")
                    for hh in range(8):
                        nc.tensor.matmul(
                            pf[:],
                            lhsT=OT_all[:, hh * spad + sj * 128: hh * spad + (sj + 1) * 128],
                            rhs=wo_sb[:, hh * DIM + oc * 512: hh * DIM + (oc + 1) * 512],
                            start=(hh == 0), stop=(hh == 7))
                    ob = obpool.tile([128, 512], F16, tag="ob")
                    nc.scalar.copy(ob[:], pf[:])
                    nc.sync.dma_start(
                        out[sj * 128:(sj + 1) * 128, oc * 512:(oc + 1) * 512],
                        ob[:])
    nc.compile()
    return nc


def _rot_matrix():
    rotT = np.zeros((HD, HD), dtype=np.float16)
    for i in range(HD // 2):
        rotT[2 * i + 1, 2 * i] = -1.0
        rotT[2 * i, 2 * i + 1] = 1.0
    return rotT


def _host_shards(x, freqs_cos, freqs_sin, vis_mask, wqkv, wo, spad=SPAD):
    x = np.asarray(x, dtype=np.float32)
    freqs_cos = np.asarray(freqs_cos, dtype=np.float32)
    freqs_sin = np.asarray(freqs_sin, dtype=np.float32)
    vis = np.asarray(vis_mask).astype(bool)
    wqkv = np.asarray(wqkv, dtype=np.float32)
    wo = np.asarray(wo, dtype=np.float32)
    nt = spad // 128
    rotT = _rot_matrix()

    # per-head-group weights (shared by cores with the same g)
    wmats = []
    for g in range(2):
        wq = wqkv[g * CC:(g + 1) * CC]
        wk = wqkv[DIM + g * CC: DIM + (g + 1) * CC]
        wqk_full = np.concatenate([wq, wk], axis=0)  # [2048 ch, 2048 dim]
        wqk_t = np.ascontiguousarray(
            wqk_full.T.reshape(16, 128, 16, 128).transpose(1, 2, 0, 3)
            .reshape(128, 16 * 16 * 128)).astype(np.float16)
        wv_g = wqkv[2 * DIM + g * CC: 2 * DIM + (g + 1) * CC]  # [1024, 2048]
        wv_t = np.ascontiguousarray(
            wv_g.T.reshape(16, 128, CC).transpose(1, 0, 2)
            .reshape(128, 16 * CC)).astype(np.float16)
        wo_g = wo[:, g * CC:(g + 1) * CC]  # [2048 out, 1024 d]
        wo_t = np.ascontiguousarray(
            wo_g.T.reshape(8, 128, DIM).transpose(1, 0, 2)
            .reshape(128, 8 * DIM)).astype(np.float16)
        wmats.append((wqk_t, wv_t, wo_t))

    # per-batch gathered tensors (shared by cores with the same b)
    bmats = []
    for b in range(B):
        idx = np.nonzero(vis[b])[0]
        sv = len(idx)
        assert sv <= spad
        xp = np.zeros((spad, DIM), dtype=np.float32)
        xp[:sv] = x[b][idx]
        xg = np.ascontiguousarray(
            xp.T.reshape(16, 128, spad).transpose(1, 0, 2)
            .reshape(128, 16 * spad)).astype(np.float16)
        cp = np.zeros((spad, HD), dtype=np.float32)
        cp[:sv] = freqs_cos[0, idx, 0, :]
        sp = np.zeros((spad, HD), dtype=np.float32)
        sp[:sv] = freqs_sin[0, idx, 0, :]
        cosg = np.ascontiguousarray(cp.T).astype(np.float16)
        sing = np.ascontiguousarray(sp.T).astype(np.float16)
        padcv = np.float32((spad - sv) * math.exp(EXP_BIAS))
        padc = np.full((1, 1), padcv, dtype=np.float32)
        bmats.append((xg, cosg, sing, padc))

    in_maps = []
    for c in range(NC):
        b, g = c // 2, c % 2
        wqk_t, wv_t, wo_t = wmats[g]
        xg, cosg, sing, padc = bmats[b]
        in_maps.append({
            "xg": xg, "wqk": wqk_t, "wv": wv_t, "wo": wo_t,
            "cosg": cosg, "sing": sing, "padc": padc, "rotT": rotT,
        })
    return in_maps


def _numpy_fallback(x, freqs_cos, freqs_sin, vis_mask, wqkv, wo):
    # exact reference math; only used if a batch has > SPAD visible rows
    # (impossible for Bernoulli(0.5) masks, kept for safety)
    x = np.asarray(x, dtype=np.float32)
    fc = np.asarray(freqs_cos, dtype=np.float32)
    fs = np.asarray(freqs_sin, dtype=np.float32)
    vis = np.asarray(vis_mask).astype(bool)
    wqkv = np.asarray(wqkv, dtype=np.float32)
    wo = np.asarray(wo, dtype=np.float32)
    qkv = np.einsum('bsd,od->bso', x, wqkv)
    xq, xk, xv = np.split(qkv, 3, axis=-1)
    xq = xq.reshape(B, S, H, HD)
    xk = xk.reshape(B, S, H, HD)
    xv = xv.reshape(B, S, H, HD)

    def rot(t):
        t2 = t.reshape(t.shape[:-1] + (-1, 2))
        r = np.stack([-t2[..., 1], t2[..., 0]], axis=-1)
        return r.reshape(t.shape)

    xq = xq * fc + rot(xq) * fs
    xk = xk * fc + rot(xk) * fs
    s = np.einsum('bqhd,bkhd->bhqk', xq, xk) * SM_SCALE
    am = (vis[:, None, :, None] & vis[:, None, None, :])
    s = np.where(am, s, -np.inf)
    m = np.maximum(np.max(s, axis=-1, keepdims=True), np.float32(-1e20))
    p = np.where(am, np.exp(s - m), 0.0)
    denom = np.maximum(np.sum(p, axis=-1, keepdims=True), np.float32(1e-6))
    attn = p / denom
    o = np.einsum('bhqk,bkhd->bqhd', attn, xv).reshape(B, S, DIM)
    return np.einsum('bsd,od->bso', o, wo).astype(np.float32)


def kernel(x, freqs_cos, freqs_sin, vis_mask, wqkv, wo):
    vis = np.asarray(vis_mask).astype(bool)
    svs = [int(vis[b].sum()) for b in range(B)]
    if max(svs) > SPAD:
        return _numpy_fallback(x, freqs_cos, freqs_sin, vis_mask, wqkv, wo)

    if "nc" not in _CACHE:
        _CACHE["nc"] = _build_program(SPAD)
    nc = _CACHE["nc"]
    in_maps = _host_shards(x, freqs_cos, freqs_sin, vis_mask, wqkv, wo)
    res = run_bass_kernel_spmd(nc, in_maps, core_ids=list(range(NC)))
    outs = [r["out"] for r in res.results]  # [SPAD, DIM] f16 partials
    final = np.zeros((B, S, DIM), dtype=np.float32)
    for b in range(B):
        idx = np.nonzero(vis[b])[0]
        sv = len(idx)
        final[b][idx] = (outs[2 * b][:sv].astype(np.float32)
                         + outs[2 * b + 1][:sv].astype(np.float32))
    return final


# revision 27
# speedup vs baseline: 1.0084x; 1.0084x over previous
"""Trainium2 Bass kernel for nn_Attention_20976620274235 (sparse attention).

Key idea: vis_mask rows/cols that are masked out contribute exactly zero to
the output (masked q rows give attn=0 -> out row 0; masked k positions are
excluded from the softmax).  So we COMPACT: host gathers the ~S/2 visible
positions per batch, pads to SPAD=1152 (=9*128, +5.7 sigma above the
Binomial(2048,.5) mean), the device computes attention on the short
sequence, and the host scatters rows back (zeros elsewhere).

Sharding: 8 cores = 4 batches x 2 head-groups (8 heads each).
Per-core SPMD program (all fp16 matmuls, fp32 PSUM):
  1. k-head projections + RoPE (q/k SBUF-resident, head-dim-major [hd, s])
  2. V projection (s-major fp16)
  3. per head h: q-head projection, then attention with TRANSPOSED scores
     sT[k, q] = kT.T @ qT so P^T feeds P@V directly as the moving operand.
     Masking needs no bias: padded K rows are exactly zero, so their
     exp(0*scale + EXP_BIAS) contribution to the softmax denominator is
     the host-known constant n_pad*e^EXP_BIAS, subtracted exactly
     (padded V rows are zero, so P@V is unaffected; padded q rows are
     discarded by the host scatter).  Scores run two k-tiles ahead of
     PV/denominator so the scalar-engine Exp latency stays off the PE
     critical path; the 128-wide tail q-chunk batches 3 k-tiles per Exp.
     Unnormalized accumulate; divide by (ones^T @ P^T - padcorr) at the
     end.  Interleaving per-head QKV with attention gives the scalar
     engine PE-only stretches to catch up in.
  4. output projection, partial over this core's 1024 channels.
Host: sums the two head-group partials per batch, scatters visible rows.

Engine assignment: PE matmuls; scalar engine does Exp (and PSUM copies
only in attention-free stretches); DVE does RoPE muls/adds, reciprocal,
final scale.  One PSUM plan for the whole kernel (8 banks): shared
scores/projection pool (3) + po (2) + pd (2) + rot/broadcast (1).
"""

import math

import numpy as np

import concourse.bass as bass
from concourse import bacc
import concourse.mybir as mybir
import concourse.tile as tile
from concourse.bass_utils import run_bass_kernel_spmd

B, S, DIM, H = 4, 2048, 2048, 16
HD = 128          # head dim
NC = 8            # cores
HC = 8            # heads per core
CC = HC * HD      # 1024 channels per core
SPAD = 1152       # padded compacted sequence length (9 * 128)
F32 = mybir.dt.float32
F16 = mybir.dt.float16
SM_SCALE = 1.0 / math.sqrt(HD)
EXP_BIAS = -6.0   # shift-invariant; keeps exp() in f16 normal range
MASK_BIAS = -1.0e9

_CACHE = {}


def _build_program(spad):
    assert spad == 1152
    nt = spad // 128   # 9 k-tiles
    TQW = spad - 1024  # 128-wide tail q-chunk

    nc = bacc.Bacc("TRN2", target_bir_lowering=False, debug=False, num_devices=NC)

    # host-pretiled inputs: layouts match SBUF exactly (contiguous DMAs)
    xg = nc.dram_tensor("xg", [128, 16 * spad], F16, kind="ExternalInput").ap()
    wqk = nc.dram_tensor("wqk", [128, 16 * 16 * 128], F16, kind="ExternalInput").ap()
    wv = nc.dram_tensor("wv", [128, 16 * CC], F16, kind="ExternalInput").ap()
    wo = nc.dram_tensor("wo", [128, 8 * DIM], F16, kind="ExternalInput").ap()
    cosg = nc.dram_tensor("cosg", [HD, spad], F16, kind="ExternalInput").ap()
    sing = nc.dram_tensor("sing", [HD, spad], F16, kind="ExternalInput").ap()
    padc = nc.dram_tensor("padc", [1, 1], F32, kind="ExternalInput").ap()
    rotT = nc.dram_tensor("rotT", [HD, HD], F16, kind="ExternalInput").ap()
    out = nc.dram_tensor("out", [spad, DIM], F16, kind="ExternalOutput").ap()

    Exp = mybir.ActivationFunctionType.Exp

    with tile.TileContext(nc) as tc:
        with tc.tile_pool(name="consts", bufs=1) as cpool, \
             tc.tile_pool(name="persist", bufs=1) as ppool, \
             tc.tile_pool(name="xp", bufs=1) as xpool, \
             tc.tile_pool(name="qc", bufs=2) as qpool, \
             tc.tile_pool(name="wmp", bufs=2) as wmpool, \
             tc.tile_pool(name="rp", bufs=2) as rpool, \
             tc.tile_pool(name="ptp", bufs=4) as ptpool, \
             tc.tile_pool(name="smp", bufs=1) as smpool, \
             tc.tile_pool(name="obp", bufs=2) as obpool, \
             tc.tile_pool(name="pss", bufs=3, space="PSUM") as pss, \
             tc.tile_pool(name="pso", bufs=2, space="PSUM") as pso, \
             tc.tile_pool(name="psd", bufs=2, space="PSUM") as psd, \
             tc.tile_pool(name="psb", bufs=1, space="PSUM") as psb:
            cos_sb = cpool.tile([HD, spad], F16)
            sin_sb = cpool.tile([HD, spad], F16)
            pc_sb = cpool.tile([1, 1], F32)
            rt_sb = cpool.tile([HD, HD], F16)
            ones_sb = cpool.tile([128, 1], F16)
            onesr_sb = cpool.tile([1, 128], F16)
            eb_sb = cpool.tile([128, 1], F32)
            nc.gpsimd.memset(ones_sb[:], 1.0)
            nc.gpsimd.memset(onesr_sb[:], 1.0)
            nc.gpsimd.memset(eb_sb[:], EXP_BIAS)

            k_all = ppool.tile([128, 8 * spad], F16)    # [hd, kh*spad + pos]
            V_all = ppool.tile([128, nt * CC], F16)     # [s%128, j*CC + ch]
            OT_all = ppool.tile([128, HC * spad], F16)  # [hd, h*spad + pos]

            # x chunked per contraction tile; DMA only the first few chunks
            # before the first weight tile so nothing big blocks it
            x_t = []
            for t in range(16):
                xt = xpool.tile([128, spad], F16, tag=f"x{t}")
                x_t.append(xt)
            for t in range(3):
                nc.sync.dma_start(x_t[t][:], xg[:, t * spad:(t + 1) * spad])

            # RoPE chain runs one (m, chunk) behind the projection matmuls
            pending = [None]

            def flush_rope():
                if pending[0] is None:
                    return
                ps, dst, d0, c0, cw = pending[0]
                pending[0] = None
                qraw = rpool.tile([128, 512], F16, tag="qraw")
                nc.scalar.copy(qraw[:, :cw], ps[:, :cw])
                pr = psb.tile([128, 512], F32, tag="bc")
                nc.tensor.matmul(pr[:, :cw], lhsT=rt_sb[:], rhs=qraw[:, :cw],
                                 start=True, stop=True)
                t1 = rpool.tile([128, 512], F16, tag="t1")
                nc.vector.tensor_mul(t1[:, :cw], qraw[:, :cw],
                                     cos_sb[:, c0:c0 + cw])
                t2 = rpool.tile([128, 512], F16, tag="t2")
                nc.vector.tensor_mul(t2[:, :cw], pr[:, :cw],
                                     sin_sb[:, c0:c0 + cw])
                nc.vector.tensor_add(dst[:, d0:d0 + cw], t1[:, :cw], t2[:, :cw])

            def qk_project(m, dst, chunk_order, wm=None):
                if wm is None:
                    wm = wmpool.tile([128, 16 * 128], F16, tag="wm")
                    nc.sync.dma_start(wm[:], wqk[:, m * 2048:(m + 1) * 2048])
                for c0, cw in chunk_order:
                    ps = pss.tile([128, 512], F32, tag="sps")
                    for t in range(16):
                        nc.tensor.matmul(
                            ps[:, :cw],
                            lhsT=wm[:, t * 128:(t + 1) * 128],
                            rhs=x_t[t][:, c0:c0 + cw],
                            start=(t == 0), stop=(t == 15))
                    flush_rope()
                    pending[0] = (ps, dst, c0, c0, cw)

            CHUNKS = [(0, 512), (512, 512), (1024, TQW)]

            # ---- k-head projections (m 8..15), then V ----
            # weight tiles for m=8,9 queued before the bulk x DMAs; all x
            # DMAs are emitted before any matmul that reads them
            wm8 = wmpool.tile([128, 16 * 128], F16, tag="wm")
            nc.sync.dma_start(wm8[:], wqk[:, 8 * 2048: 9 * 2048])
            wm9 = wmpool.tile([128, 16 * 128], F16, tag="wm")
            nc.sync.dma_start(wm9[:], wqk[:, 9 * 2048: 10 * 2048])
            nc.sync.dma_start(rt_sb[:], rotT[:])
            nc.sync.dma_start(cos_sb[:], cosg[:])
            nc.sync.dma_start(sin_sb[:], sing[:])
            nc.sync.dma_start(pc_sb[:], padc[:])
            for t in range(3, 16):
                nc.sync.dma_start(x_t[t][:], xg[:, t * spad:(t + 1) * spad])
            qk_project(8, k_all[:, 0:spad], CHUNKS, wm=wm8)
            qk_project(9, k_all[:, spad:2 * spad], CHUNKS, wm=wm9)
            with tc.tile_pool(name="wvp", bufs=1) as wvpool:
                wv_sb = wvpool.tile([128, 16 * CC], F16)
                nc.sync.dma_start(wv_sb[:], wv[:])
                for m in range(10, 16):
                    qk_project(m, k_all[:, (m - 8) * spad:(m - 7) * spad], CHUNKS)
                for j in range(nt):  # V: out [pos, vch] s-major
                    for half in range(2):
                        pv = pss.tile([128, 512], F32, tag="sps")
                        for t in range(16):
                            nc.tensor.matmul(
                                pv[:],
                                lhsT=x_t[t][:, j * 128:(j + 1) * 128],
                                rhs=wv_sb[:, t * CC + half * 512: t * CC + (half + 1) * 512],
                                start=(t == 0), stop=(t == 15))
                        nc.scalar.copy(
                            V_all[:, j * CC + half * 512: j * CC + (half + 1) * 512],
                            pv[:])
            wo_sb = ppool.tile([128, 8 * DIM], F16)
            nc.sync.dma_start(wo_sb[:], wo[:])  # prefetch for output proj

            # ---- per head: q projection then attention ----
            for h in range(HC):
                q_t = qpool.tile([128, spad], F16, tag="qcur")
                # attention consumes c0 first; c1 is flushed at loop end
                qk_project(h, q_t, [CHUNKS[2], CHUNKS[0], CHUNKS[1]])
                flush_rope()
                kbase = h * spad

                def kT(t):
                    return k_all[:, kbase + t * 128: kbase + (t + 1) * 128]

                def vT(t):
                    return V_all[:, t * CC + h * 128: t * CC + (h + 1) * 128]

                def normalize(po, pd, c0, cw):
                    den = smpool.tile([1, 512], F32, tag="den")
                    nc.vector.tensor_scalar_sub(den[:, :cw], pd[0:1, :cw],
                                                pc_sb[:])
                    rec = smpool.tile([1, 512], F32, tag="rec")
                    nc.vector.reciprocal_approx_fast(rec[:, :cw], den[:, :cw])
                    rec16 = smpool.tile([1, 512], F16, tag="rec16")
                    nc.vector.tensor_copy(rec16[:, :cw], rec[:, :cw])
                    bcp = psb.tile([128, 512], F32, tag="bc")
                    nc.tensor.matmul(bcp[:, :cw], lhsT=onesr_sb[:],
                                     rhs=rec16[:, :cw], start=True, stop=True)
                    bcs = smpool.tile([128, 512], F16, tag="bcs")
                    nc.vector.tensor_copy(bcs[:, :cw], bcp[:, :cw])
                    nc.vector.tensor_mul(
                        OT_all[:, h * spad + c0: h * spad + c0 + cw],
                        po[:, :cw], bcs[:, :cw])

                # 512-wide q-chunks: per-t exp, scores 2 k-tiles ahead
                for c0, cw in CHUNKS[:2]:
                    po = pso.tile([128, 512], F32, tag="po")
                    pd = psd.tile([128, 512], F32, tag="pd")
                    pt_l = {}
                    for t in range(nt + 2):
                        if t < nt:
                            sp = pss.tile([128, 512], F32, tag="sps")
                            nc.tensor.matmul(
                                sp[:, :cw], lhsT=kT(t),
                                rhs=q_t[:, c0:c0 + cw],
                                start=True, stop=True)
                            pt = ptpool.tile([128, 512], F16, tag="pt")
                            nc.scalar.activation(pt[:, :cw], sp[:, :cw], Exp,
                                                 bias=eb_sb[:], scale=SM_SCALE)
                            pt_l[t] = pt
                        tt = t - 2
                        if 0 <= tt:
                            pt = pt_l.pop(tt)
                            nc.tensor.matmul(
                                po[:, :cw], lhsT=vT(tt), rhs=pt[:, :cw],
                                start=(tt == 0), stop=(tt == nt - 1))
                            nc.tensor.matmul(
                                pd[0:1, :cw], lhsT=ones_sb[:], rhs=pt[:, :cw],
                                start=(tt == 0), stop=(tt == nt - 1))
                    normalize(po, pd, c0, cw)

                # tail q-chunk (128 wide): batch 3 k-tiles per exp
                c0, cw = CHUNKS[2]
                po = pso.tile([128, 512], F32, tag="po")
                pd = psd.tile([128, 512], F32, tag="pd")
                pt_l = {}
                for g in range(4):  # groups of 3 k-tiles; one-ahead pipeline
                    if g < 3:
                        sp = pss.tile([128, 512], F32, tag="sps")
                        for i in range(3):
                            t = g * 3 + i
                            nc.tensor.matmul(
                                sp[:, i * 128:(i + 1) * 128], lhsT=kT(t),
                                rhs=q_t[:, c0:c0 + cw],
                                start=True, stop=True)
                        pt = ptpool.tile([128, 512], F16, tag="pt")
                        nc.scalar.activation(pt[:, :384], sp[:, :384], Exp,
                                             bias=eb_sb[:], scale=SM_SCALE)
                        pt_l[g] = pt
                    gg = g - 1
                    if 0 <= gg:
                        pt = pt_l.pop(gg)
                        for i in range(3):
                            t = gg * 3 + i
                            nc.tensor.matmul(
                                po[:, :cw], lhsT=vT(t),
                                rhs=pt[:, i * 128:(i + 1) * 128],
                                start=(t == 0), stop=(t == nt - 1))
                            nc.tensor.matmul(
                                pd[0:1, :cw], lhsT=ones_sb[:],
                                rhs=pt[:, i * 128:(i + 1) * 128],
                                start=(t == 0), stop=(t == nt - 1))
                normalize(po, pd, c0, cw)

            # ---- output projection ----
            for sj in range(nt):
                for oc in range(4):
                    pf = pss.tile([128, 512], F32, tag="sps")
                    for hh in range(8):
                        nc.tensor.matmul(
                            pf[:],
                            lhsT=OT_all[:, hh * spad + sj * 128: hh * spad + (sj + 1) * 128],
                            rhs=wo_sb[:, hh * DIM + oc * 512: hh * DIM + (oc + 1) * 512],
                            start=(hh == 0), stop=(hh == 7))
                    ob = obpool.tile([128, 512], F16, tag="ob")
                    nc.scalar.copy(ob[:], pf[:])
                    nc.sync.dma_start(
                        out[sj * 128:(sj + 1) * 128, oc * 512:(oc + 1) * 512],
                        ob[:])
    nc.compile()
    return nc


def _rot_matrix():
    rotT = np.zeros((HD, HD), dtype=np.float16)
    for i in range(HD // 2):
        rotT[2 * i + 1, 2 * i] = -1.0
        rotT[2 * i, 2 * i + 1] = 1.0
    return rotT


def _host_shards(x, freqs_cos, freqs_sin, vis_mask, wqkv, wo, spad=SPAD):
    x = np.asarray(x, dtype=np.float32)
    freqs_cos = np.asarray(freqs_cos, dtype=np.float32)
    freqs_sin = np.asarray(freqs_sin, dtype=np.float32)
    vis = np.asarray(vis_mask).astype(bool)
    wqkv = np.asarray(wqkv, dtype=np.float32)
    wo = np.asarray(wo, dtype=np.float32)
    nt = spad // 128
    rotT = _rot_matrix()

    # per-head-group weights (shared by cores with the same g)
    wmats = []
    for g in range(2):
        wq = wqkv[g * CC:(g + 1) * CC]
        wk = wqkv[DIM + g * CC: DIM + (g + 1) * CC]
        wqk_full = np.concatenate([wq, wk], axis=0)  # [2048 ch, 2048 dim]
        wqk_t = np.ascontiguousarray(
            wqk_full.T.reshape(16, 128, 16, 128).transpose(1, 2, 0, 3)
            .reshape(128, 16 * 16 * 128)).astype(np.float16)
        wv_g = wqkv[2 * DIM + g * CC: 2 * DIM + (g + 1) * CC]  # [1024, 2048]
        wv_t = np.ascontiguousarray(
            wv_g.T.reshape(16, 128, CC).transpose(1, 0, 2)
            .reshape(128, 16 * CC)).astype(np.float16)
        wo_g = wo[:, g * CC:(g + 1) * CC]  # [2048 out, 1024 d]
        wo_t = np.ascontiguousarray(
            wo_g.T.reshape(8, 128, DIM).transpose(1, 0, 2)
            .reshape(128, 8 * DIM)).astype(np.float16)
        wmats.append((wqk_t, wv_t, wo_t))

    # per-batch gathered tensors (shared by cores with the same b)
    bmats = []
    for b in range(B):
        idx = np.nonzero(vis[b])[0]
        sv = len(idx)
        assert sv <= spad
        xp = np.zeros((spad, DIM), dtype=np.float32)
        xp[:sv] = x[b][idx]
        xg = np.ascontiguousarray(
            xp.T.reshape(16, 128, spad).transpose(1, 0, 2)
            .reshape(128, 16 * spad)).astype(np.float16)
        cp = np.zeros((spad, HD), dtype=np.float32)
        cp[:sv] = freqs_cos[0, idx, 0, :]
        sp = np.zeros((spad, HD), dtype=np.float32)
        sp[:sv] = freqs_sin[0, idx, 0, :]
        cosg = np.ascontiguousarray(cp.T).astype(np.float16)
        sing = np.ascontiguousarray(sp.T).astype(np.float16)
        padcv = np.float32((spad - sv) * math.exp(EXP_BIAS))
        padc = np.full((1, 1), padcv, dtype=np.float32)
        bmats.append((xg, cosg, sing, padc))

    in_maps = []
    for c in range(NC):
        b, g = c // 2, c % 2
        wqk_t, wv_t, wo_t = wmats[g]
        xg, cosg, sing, padc = bmats[b]
        in_maps.append({
            "xg": xg, "wqk": wqk_t, "wv": wv_t, "wo": wo_t,
            "cosg": cosg, "sing": sing, "padc": padc, "rotT": rotT,
        })
    return in_maps


def _numpy_fallback(x, freqs_cos, freqs_sin, vis_mask, wqkv, wo):
    # exact reference math; only used if a batch has > SPAD visible rows
    # (impossible for Bernoulli(0.5) masks, kept for safety)
    x = np.asarray(x, dtype=np.float32)
    fc = np.asarray(freqs_cos, dtype=np.float32)
    fs = np.asarray(freqs_sin, dtype=np.float32)
    vis = np.asarray(vis_mask).astype(bool)
    wqkv = np.asarray(wqkv, dtype=np.float32)
    wo = np.asarray(wo, dtype=np.float32)
    qkv = np.einsum('bsd,od->bso', x, wqkv)
    xq, xk, xv = np.split(qkv, 3, axis=-1)
    xq = xq.reshape(B, S, H, HD)
    xk = xk.reshape(B, S, H, HD)
    xv = xv.reshape(B, S, H, HD)

    def rot(t):
        t2 = t.reshape(t.shape[:-1] + (-1, 2))
        r = np.stack([-t2[..., 1], t2[..., 0]], axis=-1)
        return r.reshape(t.shape)

    xq = xq * fc + rot(xq) * fs
    xk = xk * fc + rot(xk) * fs
    s = np.einsum('bqhd,bkhd->bhqk', xq, xk) * SM_SCALE
    am = (vis[:, None, :, None] & vis[:, None, None, :])
    s = np.where(am, s, -np.inf)
    m = np.maximum(np.max(s, axis=-1, keepdims=True), np.float32(-1e20))
    p = np.where(am, np.exp(s - m), 0.0)
    denom = np.maximum(np.sum(p, axis=-1, keepdims=True), np.float32(1e-6))
    attn = p / denom
    o = np.einsum('bhqk,bkhd->bqhd', attn, xv).reshape(B, S, DIM)
    return np.einsum('bsd,od->bso', o, wo).astype(np.float32)


def kernel(x, freqs_cos, freqs_sin, vis_mask, wqkv, wo):
    vis = np.asarray(vis_mask).astype(bool)
    svs = [int(vis[b].sum()) for b in range(B)]
    if max(svs) > SPAD:
        return _numpy_fallback(x, freqs_cos, freqs_sin, vis_mask, wqkv, wo)

    if "nc" not in _CACHE:
        _CACHE["nc"] = _build_program(SPAD)
    nc = _CACHE["nc"]
    in_maps = _host_shards(x, freqs_cos, freqs_sin, vis_mask, wqkv, wo)
    res = run_bass_kernel_spmd(nc, in_maps, core_ids=list(range(NC)))
    outs = [r["out"] for r in res.results]  # [SPAD, DIM] f16 partials
    final = np.zeros((B, S, DIM), dtype=np.float32)
    for b in range(B):
        idx = np.nonzero(vis[b])[0]
        sv = len(idx)
        final[b][idx] = (outs[2 * b][:sv].astype(np.float32)
                         + outs[2 * b + 1][:sv].astype(np.float32))
    return final
